# revision 24
# baseline (speedup 1.0000x reference)
"""Trainium2 Bass kernel for CausalBiasingNetwork bias computation.

bias[b,s,t] = sum_r (hs[b,s]@wc_r + bc_r)*strength_r * (hs[b,t]@we_r)
             + hs[b,t] @ be.sum(0)

Folded into a rank-17 form: append rule r=16 with wc=0, bc=1, strength=1,
we=be.sum(0).  Then with
    scaledT[r,s] = (hs[b,s] @ wc'_r + bc'_r) * strength'_r      [17, S]
    uT[r,t]     = hs[b,t] @ we'_r                               [17, S]
    bias[b]     = scaledT.T @ uT                                [S, S]

Sharding (sequence-parallel, per the hint): 8 cores = 4 batches x 2
sequence halves; each device computes bias[:, s_shard, :] from a local
slice of scaledT and the full uT.  The two rank-17 projections (0.05%
of the FLOPs) are computed host-side during input sharding and shipped
as bf16 inputs (1.5 MB/core); the device runs the 2.3 TFLOP bias
matmul and the 16.8 MB f16 store stream, which is the memory roofline.

The kernel is store-stream-bound: 16.8 MB of f16 output leaves via the
16 SDMA engines at ~410 GB/s, and the slowest engine (SDMA 15, ~14%
slower, a known TRN2 quirk) needs ~46.6 us for its 1/16 share.  Total
time = (time until block 0's first store is ready) + ~46.6 us +
~2.6 us framework teardown, so the whole design minimizes the
production ramp:

- Work is cut into 16 1 MB blocks of 2 s-tiles x one t-half.  Per
  t-group j a pair of K=17 matmuls (PE strips via tile_position,
  alternating 0/32 and 64/96 between consecutive blocks so next-block
  weight loads overlap current-block matmuls) fills one 2-bank psum
  tile, which one engine drains with a single 1024-wide copy (strided
  destination AP over the two s-tile column blocks of the osb tile).
  One block = 4 psum tiles = the whole 8-bank PSUM.
- Drains alternate vector/scalar (the only two engines with PSUM
  access); together they produce ~1 MB / 2.26 us, slightly above the
  stream's consumption, so the stream never starves after the ramp.
- Each block stores with one DMA of 4 KB-contiguous chunks on the sync
  ring; the 8-deep osb pool (8 MB) decouples drains from the ~2 us
  store-completion receipt latency.
- Input loads are split across both HWDGE rings, need-ordered, with a
  small lead slice per ring (st cols 0:256, ut cols 0:512) so the
  first matmul starts ~2 us before the bulk slices land; block 0 is
  emitted as two single-j micro-blocks plus a half block so the first
  store issues after ONE drain.

Output columns are stored local-half-first; the host unrolls them when
assembling the full [4, 4096, 4096] output.
"""

import contextlib

import ml_dtypes
import numpy as np

import concourse.bacc as bacc
import concourse.bass as bass
import concourse.mybir as mybir
import concourse.tile as tile
from concourse.bass_utils import run_bass_kernel_spmd

B, S, H, R = 4, 4096, 1024, 16
R1 = R + 1          # 17 rules after folding the be-bias term
SH = S // 2         # 2048 output rows per core
P = 128             # partitions
TG = 512            # t-group width (one psum bank of f32)
F32 = mybir.dt.float32
F16 = mybir.dt.float16
BF16 = mybir.dt.bfloat16


def _emit(tc, aps):
    nc = tc.nc
    st_in, ut_in, out = aps["st"], aps["ut"], aps["out"]

    with contextlib.ExitStack() as ctx:
        big_pool = ctx.enter_context(tc.tile_pool(name="big", bufs=1))
        out_pool = ctx.enter_context(tc.tile_pool(name="out", bufs=8))
        psb_pool = ctx.enter_context(
            tc.tile_pool(name="psb", bufs=4, space="PSUM"))

        # st holds s-tile pairs (blocks): block gg has s-tile 2gg at
        # partition base 64*(gg%2), cols gg*256, and s-tile 2gg+1 at
        # base +32, cols +128.  ut local-half-first, replicated at
        # partition bases 0/32/64/96.  Loads are staged need-first as
        # separate tiles so block 0's first matmuls gate on a small
        # fast lead DMA instead of the whole tensor: the lead slices
        # (st cols 0:256 = block 0's strips, ut cols 0:512 = t-group 0)
        # land ~2 us before the big trailing slices.
        st_t = [big_pool.tile([P, TG // 2], BF16, name="st0"),
                big_pool.tile([P, 4 * TG - TG // 2], BF16, name="str")]
        ut_l = [big_pool.tile([P, TG], BF16, name="utl0"),
                big_pool.tile([P, TG], BF16, name="utl1"),
                big_pool.tile([P, 2 * TG], BF16, name="utlr")]
        ut_peer = big_pool.tile([P, SH], BF16, name="utp")

        # loads split across BOTH rings, ordered by first need, with a
        # small lead slice per ring (st cols 0:256 = block 0's strips,
        # ut cols 0:512 = t-group 0) so block 0's matmuls start ~2 us
        # before the bulk slices land.  Only the partitions actually
        # read are loaded: strips live at partitions 0..112 (4 bases +
        # 17 rule rows), and the peer half uses 2 bases (partitions
        # 0..48) -- trimming ~440 KB of load traffic off the ramp.
        PL = 3 * 32 + R1      # 113: last local strip base + 17 rows
        PQ = 32 + R1          # 49: peer half uses bases 0/32 only
        nc.sync.dma_start(st_t[0][0:PL, :], st_in[0:PL, 0:TG // 2])
        nc.scalar.dma_start(ut_l[0][0:PL, :], ut_in[0:PL, 0:TG])
        nc.sync.dma_start(ut_l[1][0:PL, :], ut_in[0:PL, TG:2 * TG])
        nc.scalar.dma_start(ut_l[2][0:PL, :], ut_in[0:PL, 2 * TG:SH])
        nc.sync.dma_start(st_t[1][0:PL, :], st_in[0:PL, TG // 2:])
        nc.scalar.dma_start(ut_peer[0:PQ, :], ut_in[0:PQ, SH:])

        def st_ap(gg, b0, a):
            c = gg * 2 * P + a * P
            if gg == 0:
                return st_t[0][b0:b0 + R1, c:c + P]
            return st_t[1][b0:b0 + R1, c - 2 * P:c - P]

        def ut_ap(pr, b0, j):
            if pr == 1:
                return ut_peer[b0:b0 + R1, j * TG:(j + 1) * TG]
            if j < 2:
                return ut_l[j][b0:b0 + R1, :]
            return ut_l[2][b0:b0 + R1, (j - 2) * TG:(j - 1) * TG]

        vcopy = nc.vector.tensor_copy
        scopy = nc.scalar.copy

        def stage_blk(gg, pr, dr_eng, j_lo, j_hi):
            """Block: s-tiles {2gg, 2gg+1}, t-half pr, t-groups [j_lo,j_hi).

            Per t-group j, two K=17 matmuls (strips p0 / p0+32) fill the
            halves of one 2-bank psum tile; one engine drains it with a
            single 1024-wide copy whose destination is a strided AP
            across the two s-tile column blocks of the output tile.
            """
            # pr=1 blocks all use strip bases 0/32 (so the peer ut needs
            # only 2 replicas); by then the stream is consumer-bound, so
            # the same-base weight-load serialization is harmless
            p0 = 64 * (gg % 2) if pr == 0 else 0
            nj = j_hi - j_lo
            osb = out_pool.tile([P, 2 * nj * TG], F16, tag="o", name="osb")
            ob = osb[:]
            ppart = list(ob.ap[0])
            for j in range(j_lo, j_hi):
                pp = psb_pool.tile([P, 2 * TG], F32, tag="psb", name="pp")
                for a in range(2):
                    b0 = p0 + 32 * a
                    nc.tensor.matmul(
                        pp[:, a * TG:(a + 1) * TG],
                        st_ap(gg, b0, a),
                        ut_ap(pr, b0, j),
                        start=True, stop=True,
                        tile_position=(b0, 0),
                    )
                dst = bass.AP(
                    ob.tensor,
                    ob.offset + (j - j_lo) * TG,
                    [ppart, [nj * TG, 2], [1, TG]])
                dr_eng[j - j_lo](dst, pp[:])
            # one store for the block: DRAM AP iterates (p, a, c) to
            # match the s-tile-major osb columns; nj KB contiguous
            dst = bass.AP(
                out.tensor,
                (2 * gg * P) * S + pr * 4 * TG + j_lo * TG,
                [[S, P], [P * S, 2], [1, nj * TG]])
            nc.sync.dma_start(dst, osb[:])

        # drains alternate vector/scalar within each block.  Block 0 is
        # split into two single-j micro-blocks plus a half-width block
        # so the first store issues after ONE drain (the store stream's
        # start is the critical path; its first ~2 us are underfed
        # anyway, so small early stores cost nothing).
        VS = [vcopy, scopy, vcopy, scopy]
        SV = [scopy, vcopy, scopy, vcopy]
        stage_blk(0, 0, [vcopy], 0, 1)
        stage_blk(0, 0, [scopy], 1, 2)
        stage_blk(0, 0, VS, 2, 4)
        blocks = [(gg, pr) for pr in range(2) for gg in range(8)][1:]
        for idx, (gg, pr) in enumerate(blocks):
            stage_blk(gg, pr, SV if idx % 2 == 0 else VS, 0, 4)


def _build():
    nc = bacc.Bacc("TRN2", target_bir_lowering=False, debug=False,
                   num_devices=8)
    aps = {}
    decls = [
        ("st", [P, 4 * TG], BF16, "ExternalInput"),
        ("ut", [P, S], BF16, "ExternalInput"),
        ("out", [SH, S], F16, "ExternalOutput"),
    ]
    for name, shape, dt_, kind in decls:
        aps[name] = nc.dram_tensor(name, shape, dt_, kind=kind).ap()
    with tile.TileContext(nc) as tc:
        _emit(tc, aps)
    nc.compile()
    return nc


_CACHE = {}


def _get_nc():
    if "nc" not in _CACHE:
        _CACHE["nc"] = _build()
    return _CACHE["nc"]


def _prep_in_maps(hidden_states, wc, bc, we, be, strength):
    hsf = np.asarray(hidden_states, np.float32)
    wc = np.asarray(wc, np.float32)
    bc = np.asarray(bc, np.float32)
    we = np.asarray(we, np.float32)
    be = np.asarray(be, np.float32)
    strength = np.asarray(strength, np.float32)

    wc1 = np.concatenate([wc, np.zeros((1, H), np.float32)], 0)   # [17, H]
    bc1 = np.concatenate([bc, np.ones(1, np.float32)])
    st1 = np.concatenate([strength, np.ones(1, np.float32)])
    we1 = np.concatenate([we, be.sum(0, keepdims=True)], 0)       # [17, H]

    # host-side rank-17 projections (the "local slice of scaled and full
    # u/v" each device consumes, per the sharding hint)
    u_all = np.einsum("bsh,rh->brs", hsf, we1)                    # [B,17,S]
    scaled = (np.einsum("bsh,rh->brs", hsf, wc1)
              + bc1[None, :, None]) * st1[None, :, None]          # [B,17,S]

    in_maps = []
    for core in range(8):
        b, half = core // 2, core % 2
        # block gg = s-tile pair (2gg, 2gg+1): s-tile 2gg+a at cols
        # gg*256 + a*128, partition base 32a for every block (used by
        # pr=1 and even-gg pr=0) plus a replica at base 64+32a for odd
        # gg (pr=0 blocks alternate base sets so consecutive blocks'
        # PE weight loads overlap)
        stx = np.zeros((P, 4 * TG), np.float32)
        base = half * SH
        for gg in range(8):
            for a in range(2):
                s_tile = 2 * gg + a
                rows = scaled[b, :, base + s_tile * P:
                              base + (s_tile + 1) * P]
                cols = slice(gg * 2 * P + a * P, gg * 2 * P + (a + 1) * P)
                stx[32 * a:32 * a + R1, cols] = rows
                if gg % 2:
                    stx[64 + 32 * a:64 + 32 * a + R1, cols] = rows
        # uT in local-first column order, replicated at bases 0/32/64/96
        u_loc = np.concatenate(
            [u_all[b, :, base:base + SH],
             u_all[b, :, (1 - half) * SH:(2 - half) * SH]], axis=1)
        ut = np.zeros((P, S), np.float32)
        for i in range(4):
            ut[32 * i:32 * i + R1, :] = u_loc
        in_maps.append({
            "st": np.ascontiguousarray(stx.astype(ml_dtypes.bfloat16)),
            "ut": np.ascontiguousarray(ut.astype(ml_dtypes.bfloat16)),
        })
    return in_maps


def _assemble(results):
    full = np.empty((B, S, S), np.float32)
    for core in range(8):
        b, half = core // 2, core % 2
        o = results[core]["out"].astype(np.float32)
        if half == 0:
            full[b, :SH, :] = o
        else:
            full[b, SH:, SH:] = o[:, :SH]
            full[b, SH:, :SH] = o[:, SH:]
    return full


def kernel(hidden_states, wc, bc, we, be, strength):
    nc = _get_nc()
    in_maps = _prep_in_maps(hidden_states, wc, bc, we, be, strength)
    res = run_bass_kernel_spmd(nc, in_maps, core_ids=list(range(8)))
    return _assemble(res.results)


def kernel_traced(hidden_states, wc, bc, we, be, strength, key=None,
                  **trace_kwargs):
    """Test-harness entry: returns (output, BassKernelResults with trace)."""
    nc = _get_nc()
    in_maps = _prep_in_maps(hidden_states, wc, bc, we, be, strength)
    res = run_bass_kernel_spmd(nc, in_maps, core_ids=list(range(8)),
                               trace=True, **trace_kwargs)
    return _assemble(res.results), res


# revision 26
# speedup vs baseline: 1.6081x; 1.6081x over previous
"""Trainium2 Bass kernel for CausalBiasingNetwork bias computation.

bias[b,s,t] = sum_r (hs[b,s]@wc_r + bc_r)*strength_r * (hs[b,t]@we_r)
             + hs[b,t] @ be.sum(0)

Folded into a rank-17 form: append rule r=16 with wc=0, bc=1, strength=1,
we=be.sum(0).  Then with
    scaledT[r,s] = (hs[b,s] @ wc'_r + bc'_r) * strength'_r      [17, S]
    uT[r,t]     = hs[b,t] @ we'_r                               [17, S]
    bias[b]     = scaledT.T @ uT                                [S, S]

Sharding (sequence-parallel, per the hint): 8 cores = 4 batches x 2
sequence halves; each device computes bias[:, s_shard, :] from a local
slice of scaledT and the full uT.  The two rank-17 projections (0.05%
of the FLOPs) are computed host-side during input sharding and shipped
as bf16 inputs (1.5 MB/core); the device runs the 2.3 TFLOP bias
matmul and the 16.8 MB f16 store stream, which is the memory roofline.

The kernel is store-stream-bound: 16.8 MB of f16 output leaves via the
16 SDMA engines at ~410 GB/s, and the slowest engine (SDMA 15, ~14%
slower, a known TRN2 quirk) needs ~46.6 us for its 1/16 share.  Total
time = (time until block 0's first store is ready) + ~46.6 us +
~2.6 us framework teardown, so the whole design minimizes the
production ramp:

- Work is cut into 16 1 MB blocks of 2 s-tiles x one t-half.  Per
  t-group j a pair of K=17 matmuls (PE strips via tile_position,
  alternating 0/32 and 64/96 between consecutive blocks so next-block
  weight loads overlap current-block matmuls) fills one 2-bank psum
  tile, which one engine drains with a single 1024-wide copy (strided
  destination AP over the two s-tile column blocks of the osb tile).
  One block = 4 psum tiles = the whole 8-bank PSUM.
- Drains alternate vector/scalar (the only two engines with PSUM
  access); together they produce ~1 MB / 2.26 us, slightly above the
  stream's consumption, so the stream never starves after the ramp.
- Each block stores with one DMA of 4 KB-contiguous chunks on the sync
  ring; the 8-deep osb pool (8 MB) decouples drains from the ~2 us
  store-completion receipt latency.
- Input loads are split across both HWDGE rings, need-ordered, with a
  small lead slice per ring (st cols 0:256, ut cols 0:512) so the
  first matmul starts ~2 us before the bulk slices land; block 0 is
  emitted as two single-j micro-blocks plus a half block so the first
  store issues after ONE drain.

Output columns are stored local-half-first; the host unrolls them when
assembling the full [4, 4096, 4096] output.
"""

import contextlib

import ml_dtypes
import numpy as np

import concourse.bacc as bacc
import concourse.bass as bass
import concourse.mybir as mybir
import concourse.tile as tile
from concourse.bass_utils import run_bass_kernel_spmd

B, S, H, R = 4, 4096, 1024, 16
R1 = R + 1          # 17 rules after folding the be-bias term
SH = S // 2         # 2048 output rows per core
P = 128             # partitions
TG = 512            # t-group width (one psum bank of f32)
F32 = mybir.dt.float32
F16 = mybir.dt.float16
BF16 = mybir.dt.bfloat16


def _emit(tc, aps):
    nc = tc.nc
    st_in, ut_in, out = aps["st"], aps["ut"], aps["out"]

    with contextlib.ExitStack() as ctx:
        big_pool = ctx.enter_context(tc.tile_pool(name="big", bufs=1))
        out_pool = ctx.enter_context(tc.tile_pool(name="out", bufs=8))
        psb_pool = ctx.enter_context(
            tc.tile_pool(name="psb", bufs=4, space="PSUM"))

        # st holds s-tile pairs (blocks): block gg has s-tile 2gg at
        # partition base 64*(gg%2), cols gg*256, and s-tile 2gg+1 at
        # base +32, cols +128.  ut local-half-first, replicated at
        # partition bases 0/32/64/96.  Loads are staged need-first as
        # separate tiles so block 0's first matmuls gate on a small
        # fast lead DMA instead of the whole tensor: the lead slices
        # (st cols 0:256 = block 0's strips, ut cols 0:512 = t-group 0)
        # land ~2 us before the big trailing slices.
        st_t = [big_pool.tile([P, TG // 2], BF16, name="st0"),
                big_pool.tile([P, 4 * TG - TG // 2], BF16, name="str")]
        ut_l = [big_pool.tile([P, TG], BF16, name="utl0"),
                big_pool.tile([P, TG], BF16, name="utl1"),
                big_pool.tile([P, 2 * TG], BF16, name="utlr")]
        ut_peer = big_pool.tile([P, SH], BF16, name="utp")

        # loads split across BOTH rings, ordered by first need, with a
        # small lead slice per ring (st cols 0:256 = block 0's strips,
        # ut cols 0:512 = t-group 0) so block 0's matmuls start ~2 us
        # before the bulk slices land.  All loads span the full 128
        # partitions: partial-partition DMAs measured ~5x slower (bad
        # descriptor split), far outweighing the redundant bytes.
        nc.sync.dma_start(st_t[0][:], st_in[:, 0:TG // 2])
        nc.scalar.dma_start(ut_l[0][:], ut_in[:, 0:TG])
        nc.sync.dma_start(ut_l[1][:], ut_in[:, TG:2 * TG])
        nc.scalar.dma_start(ut_l[2][:], ut_in[:, 2 * TG:SH])
        nc.sync.dma_start(st_t[1][:], st_in[:, TG // 2:])
        nc.scalar.dma_start(ut_peer[:], ut_in[:, SH:])

        def st_ap(gg, b0, a):
            c = gg * 2 * P + a * P
            if gg == 0:
                return st_t[0][b0:b0 + R1, c:c + P]
            return st_t[1][b0:b0 + R1, c - 2 * P:c - P]

        def ut_ap(pr, b0, j):
            if pr == 1:
                return ut_peer[b0:b0 + R1, j * TG:(j + 1) * TG]
            if j < 2:
                return ut_l[j][b0:b0 + R1, :]
            return ut_l[2][b0:b0 + R1, (j - 2) * TG:(j - 1) * TG]

        vcopy = nc.vector.tensor_copy
        scopy = nc.scalar.copy

        def stage_blk(gg, pr, dr_eng, j_lo, j_hi):
            """Block: s-tiles {2gg, 2gg+1}, t-half pr, t-groups [j_lo,j_hi).

            Per t-group j, two K=17 matmuls (strips p0 / p0+32) fill the
            halves of one 2-bank psum tile; one engine drains it with a
            single 1024-wide copy whose destination is a strided AP
            across the two s-tile column blocks of the output tile.
            """
            p0 = 64 * (gg % 2)
            nj = j_hi - j_lo
            osb = out_pool.tile([P, 2 * nj * TG], F16, tag="o", name="osb")
            ob = osb[:]
            ppart = list(ob.ap[0])
            for j in range(j_lo, j_hi):
                pp = psb_pool.tile([P, 2 * TG], F32, tag="psb", name="pp")
                for a in range(2):
                    b0 = p0 + 32 * a
                    nc.tensor.matmul(
                        pp[:, a * TG:(a + 1) * TG],
                        st_ap(gg, b0, a),
                        ut_ap(pr, b0, j),
                        start=True, stop=True,
                        tile_position=(b0, 0),
                    )
                dst = bass.AP(
                    ob.tensor,
                    ob.offset + (j - j_lo) * TG,
                    [ppart, [nj * TG, 2], [1, TG]])
                dr_eng[j - j_lo](dst, pp[:])
            # one store for the block: DRAM AP iterates (p, a, c) to
            # match the s-tile-major osb columns; nj KB contiguous
            dst = bass.AP(
                out.tensor,
                (2 * gg * P) * S + pr * 4 * TG + j_lo * TG,
                [[S, P], [P * S, 2], [1, nj * TG]])
            nc.sync.dma_start(dst, osb[:])

        # drains alternate vector/scalar within each block.  Block 0 is
        # split into two single-j micro-blocks plus a half-width block
        # so the first store issues after ONE drain (the store stream's
        # start is the critical path; its first ~2 us are underfed
        # anyway, so small early stores cost nothing).
        VS = [vcopy, scopy, vcopy, scopy]
        SV = [scopy, vcopy, scopy, vcopy]
        stage_blk(0, 0, [vcopy], 0, 1)
        stage_blk(0, 0, [scopy], 1, 2)
        stage_blk(0, 0, VS, 2, 4)
        blocks = [(gg, pr) for pr in range(2) for gg in range(8)][1:]
        for idx, (gg, pr) in enumerate(blocks):
            stage_blk(gg, pr, SV if idx % 2 == 0 else VS, 0, 4)


def _build():
    nc = bacc.Bacc("TRN2", target_bir_lowering=False, debug=False,
                   num_devices=8)
    aps = {}
    decls = [
        ("st", [P, 4 * TG], BF16, "ExternalInput"),
        ("ut", [P, S], BF16, "ExternalInput"),
        ("out", [SH, S], F16, "ExternalOutput"),
    ]
    for name, shape, dt_, kind in decls:
        aps[name] = nc.dram_tensor(name, shape, dt_, kind=kind).ap()
    with tile.TileContext(nc) as tc:
        _emit(tc, aps)
    nc.compile()
    return nc


_CACHE = {}


def _get_nc():
    if "nc" not in _CACHE:
        _CACHE["nc"] = _build()
    return _CACHE["nc"]


def _prep_in_maps(hidden_states, wc, bc, we, be, strength):
    hsf = np.asarray(hidden_states, np.float32)
    wc = np.asarray(wc, np.float32)
    bc = np.asarray(bc, np.float32)
    we = np.asarray(we, np.float32)
    be = np.asarray(be, np.float32)
    strength = np.asarray(strength, np.float32)

    wc1 = np.concatenate([wc, np.zeros((1, H), np.float32)], 0)   # [17, H]
    bc1 = np.concatenate([bc, np.ones(1, np.float32)])
    st1 = np.concatenate([strength, np.ones(1, np.float32)])
    we1 = np.concatenate([we, be.sum(0, keepdims=True)], 0)       # [17, H]

    # host-side rank-17 projections (the "local slice of scaled and full
    # u/v" each device consumes, per the sharding hint)
    u_all = np.einsum("bsh,rh->brs", hsf, we1)                    # [B,17,S]
    scaled = (np.einsum("bsh,rh->brs", hsf, wc1)
              + bc1[None, :, None]) * st1[None, :, None]          # [B,17,S]

    in_maps = []
    for core in range(8):
        b, half = core // 2, core % 2
        # block gg = s-tile pair (2gg, 2gg+1): s-tile 2gg+a at cols
        # gg*256 + a*128, partition base 32a for every block (used by
        # pr=1 and even-gg pr=0) plus a replica at base 64+32a for odd
        # gg (pr=0 blocks alternate base sets so consecutive blocks'
        # PE weight loads overlap)
        stx = np.zeros((P, 4 * TG), np.float32)
        base = half * SH
        for gg in range(8):
            for a in range(2):
                s_tile = 2 * gg + a
                rows = scaled[b, :, base + s_tile * P:
                              base + (s_tile + 1) * P]
                cols = slice(gg * 2 * P + a * P, gg * 2 * P + (a + 1) * P)
                stx[32 * a:32 * a + R1, cols] = rows
                if gg % 2:
                    stx[64 + 32 * a:64 + 32 * a + R1, cols] = rows
        # uT in local-first column order, replicated at bases 0/32/64/96
        u_loc = np.concatenate(
            [u_all[b, :, base:base + SH],
             u_all[b, :, (1 - half) * SH:(2 - half) * SH]], axis=1)
        ut = np.zeros((P, S), np.float32)
        for i in range(4):
            ut[32 * i:32 * i + R1, :] = u_loc
        in_maps.append({
            "st": np.ascontiguousarray(stx.astype(ml_dtypes.bfloat16)),
            "ut": np.ascontiguousarray(ut.astype(ml_dtypes.bfloat16)),
        })
    return in_maps


def _assemble(results):
    full = np.empty((B, S, S), np.float32)
    for core in range(8):
        b, half = core // 2, core % 2
        o = results[core]["out"].astype(np.float32)
        if half == 0:
            full[b, :SH, :] = o
        else:
            full[b, SH:, SH:] = o[:, :SH]
            full[b, SH:, :SH] = o[:, SH:]
    return full


def kernel(hidden_states, wc, bc, we, be, strength):
    nc = _get_nc()
    in_maps = _prep_in_maps(hidden_states, wc, bc, we, be, strength)
    res = run_bass_kernel_spmd(nc, in_maps, core_ids=list(range(8)))
    return _assemble(res.results)


def kernel_traced(hidden_states, wc, bc, we, be, strength, key=None,
                  **trace_kwargs):
    """Test-harness entry: returns (output, BassKernelResults with trace)."""
    nc = _get_nc()
    in_maps = _prep_in_maps(hidden_states, wc, bc, we, be, strength)
    res = run_bass_kernel_spmd(nc, in_maps, core_ids=list(range(8)),
                               trace=True, **trace_kwargs)
    return _assemble(res.results), res


# revision 29
# speedup vs baseline: 1.6468x; 1.0241x over previous
"""Trainium2 Bass kernel for CausalBiasingNetwork bias computation.

bias[b,s,t] = sum_r (hs[b,s]@wc_r + bc_r)*strength_r * (hs[b,t]@we_r)
             + hs[b,t] @ be.sum(0)

Folded into a rank-17 form: append rule r=16 with wc=0, bc=1, strength=1,
we=be.sum(0).  Then with
    scaledT[r,s] = (hs[b,s] @ wc'_r + bc'_r) * strength'_r      [17, S]
    uT[r,t]     = hs[b,t] @ we'_r                               [17, S]
    bias[b]     = scaledT.T @ uT                                [S, S]

Sharding (sequence-parallel, per the hint): 8 cores = 4 batches x 2
sequence halves; each device computes bias[:, s_shard, :] from a local
slice of scaledT and the full uT.  The two rank-17 projections (0.05%
of the FLOPs) are computed host-side during input sharding and shipped
as bf16 inputs (1.5 MB/core); the device runs the 2.3 TFLOP bias
matmul and the 16.8 MB f16 store stream, which is the memory roofline.

The kernel is store-stream-bound: 16.8 MB of f16 output leaves via the
16 SDMA engines at ~410 GB/s, and the slowest engine (SDMA 15, ~14%
slower, a known TRN2 quirk) needs ~46.6 us for its 1/16 share.  Total
time = (time until block 0's first store is ready) + ~46.6 us +
~2.6 us framework teardown, so the whole design minimizes the
production ramp:

- Work is cut into 16 1 MB blocks of 2 s-tiles x one t-half.  Per
  t-group j a pair of K=17 matmuls (PE strips via tile_position,
  alternating 0/32 and 64/96 between consecutive blocks so next-block
  weight loads overlap current-block matmuls) fills one 2-bank psum
  tile, which one engine drains with a single 1024-wide copy (strided
  destination AP over the two s-tile column blocks of the osb tile).
  One block = 4 psum tiles = the whole 8-bank PSUM.
- Drains alternate vector/scalar (the only two engines with PSUM
  access); together they produce ~1 MB / 2.26 us, slightly above the
  stream's consumption, so the stream never starves after the ramp.
- Each block stores with one DMA of 4 KB-contiguous chunks on the sync
  ring; the 8-deep osb pool (8 MB) decouples drains from the ~2 us
  store-completion receipt latency.
- Input loads are split across both HWDGE rings, need-ordered, with a
  small lead slice per ring (st cols 0:256, ut cols 0:512) so the
  first matmul starts ~2 us before the bulk slices land; block 0 is
  emitted as two single-j micro-blocks plus a half block so the first
  store issues after ONE drain.

Output columns are stored local-half-first; the host unrolls them when
assembling the full [4, 4096, 4096] output.
"""

import contextlib

import ml_dtypes
import numpy as np

import concourse.bacc as bacc
import concourse.bass as bass
import concourse.mybir as mybir
import concourse.tile as tile
from concourse.bass_utils import run_bass_kernel_spmd

B, S, H, R = 4, 4096, 1024, 16
R1 = R + 1          # 17 rules after folding the be-bias term
SH = S // 2         # 2048 output rows per core
P = 128             # partitions
TG = 512            # t-group width (one psum bank of f32)
F32 = mybir.dt.float32
F16 = mybir.dt.float16
BF16 = mybir.dt.bfloat16


def _emit(tc, aps):
    nc = tc.nc
    st_in, ut_in, out = aps["st"], aps["ut"], aps["out"]

    with contextlib.ExitStack() as ctx:
        big_pool = ctx.enter_context(tc.tile_pool(name="big", bufs=1))
        out_pool = ctx.enter_context(tc.tile_pool(name="out", bufs=8))
        psb_pool = ctx.enter_context(
            tc.tile_pool(name="psb", bufs=4, space="PSUM"))

        # st holds s-tile pairs (blocks): block gg has s-tile 2gg at
        # partition base 64*(gg%2), cols gg*256, and s-tile 2gg+1 at
        # base +32, cols +128.  ut local-half-first, replicated at
        # partition bases 0/32/64/96.  Loads are staged need-first as
        # separate tiles so block 0's first matmuls gate on a small
        # fast lead DMA instead of the whole tensor: the lead slices
        # (st cols 0:256 = block 0's strips, ut cols 0:512 = t-group 0)
        # land ~2 us before the big trailing slices.
        st_t = [big_pool.tile([P, TG // 2], BF16, name="st0"),
                big_pool.tile([P, 4 * TG - TG // 2], BF16, name="str")]
        ut_l = [big_pool.tile([P, TG], BF16, name="utl0"),
                big_pool.tile([P, TG], BF16, name="utl1"),
                big_pool.tile([P, 2 * TG], BF16, name="utlr")]
        ut_peer = big_pool.tile([P, SH], BF16, name="utp")

        # loads split across BOTH rings, ordered by first need, with a
        # small lead slice per ring (st cols 0:256 = block 0's strips,
        # ut cols 0:512 = t-group 0) so block 0's matmuls start ~2 us
        # before the bulk slices land.  All loads span the full 128
        # partitions: partial-partition DMAs measured ~5x slower (bad
        # descriptor split), far outweighing the redundant bytes.
        nc.sync.dma_start(st_t[0][:], st_in[:, 0:TG // 2])
        nc.scalar.dma_start(ut_l[0][:], ut_in[:, 0:TG])
        nc.sync.dma_start(ut_l[1][:], ut_in[:, TG:2 * TG])
        nc.scalar.dma_start(ut_l[2][:], ut_in[:, 2 * TG:SH])
        nc.sync.dma_start(st_t[1][:], st_in[:, TG // 2:])
        nc.scalar.dma_start(ut_peer[:], ut_in[:, SH:])

        def st_ap(gg, b0, a):
            c = gg * 2 * P + a * P
            if gg == 0:
                return st_t[0][b0:b0 + R1, c:c + P]
            return st_t[1][b0:b0 + R1, c - 2 * P:c - P]

        def ut_ap(pr, b0, j):
            if pr == 1:
                return ut_peer[b0:b0 + R1, j * TG:(j + 1) * TG]
            if j < 2:
                return ut_l[j][b0:b0 + R1, :]
            return ut_l[2][b0:b0 + R1, (j - 2) * TG:(j - 1) * TG]

        vcopy = nc.vector.tensor_copy
        scopy = nc.scalar.copy

        def stage_blk(gg, pr, dr_eng, j_lo, j_hi):
            """Block: s-tiles {2gg, 2gg+1}, t-half pr, t-groups [j_lo,j_hi).

            Per t-group j, two K=17 matmuls (strips p0 / p0+32) fill the
            halves of one 2-bank psum tile; one engine drains it with a
            single 1024-wide copy whose destination is a strided AP
            across the two s-tile column blocks of the output tile.
            """
            p0 = 64 * (gg % 2)
            nj = j_hi - j_lo
            osb = out_pool.tile([P, 2 * nj * TG], F16, tag="o", name="osb")
            ob = osb[:]
            ppart = list(ob.ap[0])
            for j in range(j_lo, j_hi):
                pp = psb_pool.tile([P, 2 * TG], F32, tag="psb", name="pp")
                for a in range(2):
                    b0 = p0 + 32 * a
                    nc.tensor.matmul(
                        pp[:, a * TG:(a + 1) * TG],
                        st_ap(gg, b0, a),
                        ut_ap(pr, b0, j),
                        start=True, stop=True,
                        tile_position=(b0, 0),
                    )
                dst = bass.AP(
                    ob.tensor,
                    ob.offset + (j - j_lo) * TG,
                    [ppart, [nj * TG, 2], [1, TG]])
                dr_eng[j - j_lo](dst, pp[:])
            # one store for the block.  The DRAM layout is block-major
            # [k][p][a][c] (k = pr*8 + gg; the host unscrambles), so a
            # full block writes one 8 KB-contiguous run per partition
            # (128 descriptors/store instead of 256 4 KB ones -- halves
            # the per-packet SDMA overhead on the stream)
            k = pr * 8 + gg
            dst = bass.AP(
                out.tensor,
                k * P * S + j_lo * TG,
                [[S, P], [4 * TG, 2], [1, nj * TG]]
                if nj < 4 else
                [[S, P], [1, S]])
            nc.sync.dma_start(dst, osb[:])

        # drains alternate vector/scalar within each block.  Block 0 is
        # split into two single-j micro-blocks plus a half-width block
        # so the first store issues after ONE drain (the store stream's
        # start is the critical path; its first ~2 us are underfed
        # anyway, so small early stores cost nothing).
        VS = [vcopy, scopy, vcopy, scopy]
        SV = [scopy, vcopy, scopy, vcopy]
        stage_blk(0, 0, [vcopy], 0, 1)
        stage_blk(0, 0, [scopy], 1, 2)
        stage_blk(0, 0, VS, 2, 4)
        blocks = [(gg, pr) for pr in range(2) for gg in range(8)][1:]
        for idx, (gg, pr) in enumerate(blocks):
            stage_blk(gg, pr, SV if idx % 2 == 0 else VS, 0, 4)


def _build():
    nc = bacc.Bacc("TRN2", target_bir_lowering=False, debug=False,
                   num_devices=8)
    aps = {}
    decls = [
        ("st", [P, 4 * TG], BF16, "ExternalInput"),
        ("ut", [P, S], BF16, "ExternalInput"),
        ("out", [SH, S], F16, "ExternalOutput"),
    ]
    for name, shape, dt_, kind in decls:
        aps[name] = nc.dram_tensor(name, shape, dt_, kind=kind).ap()
    with tile.TileContext(nc) as tc:
        _emit(tc, aps)
    nc.compile()
    return nc


_CACHE = {}


def _get_nc():
    if "nc" not in _CACHE:
        _CACHE["nc"] = _build()
    return _CACHE["nc"]


def _prep_in_maps(hidden_states, wc, bc, we, be, strength):
    hsf = np.asarray(hidden_states, np.float32)
    wc = np.asarray(wc, np.float32)
    bc = np.asarray(bc, np.float32)
    we = np.asarray(we, np.float32)
    be = np.asarray(be, np.float32)
    strength = np.asarray(strength, np.float32)

    wc1 = np.concatenate([wc, np.zeros((1, H), np.float32)], 0)   # [17, H]
    bc1 = np.concatenate([bc, np.ones(1, np.float32)])
    st1 = np.concatenate([strength, np.ones(1, np.float32)])
    we1 = np.concatenate([we, be.sum(0, keepdims=True)], 0)       # [17, H]

    # host-side rank-17 projections (the "local slice of scaled and full
    # u/v" each device consumes, per the sharding hint)
    u_all = np.einsum("bsh,rh->brs", hsf, we1)                    # [B,17,S]
    scaled = (np.einsum("bsh,rh->brs", hsf, wc1)
              + bc1[None, :, None]) * st1[None, :, None]          # [B,17,S]

    in_maps = []
    for core in range(8):
        b, half = core // 2, core % 2
        # block gg = s-tile pair (2gg, 2gg+1): s-tile 2gg+a at cols
        # gg*256 + a*128, partition base 32a for every block (used by
        # pr=1 and even-gg pr=0) plus a replica at base 64+32a for odd
        # gg (pr=0 blocks alternate base sets so consecutive blocks'
        # PE weight loads overlap)
        stx = np.zeros((P, 4 * TG), np.float32)
        base = half * SH
        for gg in range(8):
            for a in range(2):
                s_tile = 2 * gg + a
                rows = scaled[b, :, base + s_tile * P:
                              base + (s_tile + 1) * P]
                cols = slice(gg * 2 * P + a * P, gg * 2 * P + (a + 1) * P)
                stx[32 * a:32 * a + R1, cols] = rows
                if gg % 2:
                    stx[64 + 32 * a:64 + 32 * a + R1, cols] = rows
        # uT in local-first column order, replicated at bases 0/32/64/96
        u_loc = np.concatenate(
            [u_all[b, :, base:base + SH],
             u_all[b, :, (1 - half) * SH:(2 - half) * SH]], axis=1)
        ut = np.zeros((P, S), np.float32)
        for i in range(4):
            ut[32 * i:32 * i + R1, :] = u_loc
        in_maps.append({
            "st": np.ascontiguousarray(stx.astype(ml_dtypes.bfloat16)),
            "ut": np.ascontiguousarray(ut.astype(ml_dtypes.bfloat16)),
        })
    return in_maps


def _assemble(results):
    full = np.empty((B, S, S), np.float32)
    for core in range(8):
        b, half = core // 2, core % 2
        # device layout is block-major [pr][gg][p][a][c]: s-tile 2gg+a,
        # row p, local-first t-half pr -- unscramble to [s, t_local]
        r = results[core]["out"].astype(np.float32)
        r = r.reshape(2, 8, P, 2, SH)
        o = np.empty((SH, S), np.float32)
        for pr in range(2):
            o.reshape(16, P, S)[:, :, pr * SH:(pr + 1) * SH] = (
                r[pr].reshape(8, P, 2, SH).transpose(0, 2, 1, 3)
                .reshape(16, P, SH))
        if half == 0:
            full[b, :SH, :] = o
        else:
            full[b, SH:, SH:] = o[:, :SH]
            full[b, SH:, :SH] = o[:, SH:]
    return full


def kernel(hidden_states, wc, bc, we, be, strength):
    nc = _get_nc()
    in_maps = _prep_in_maps(hidden_states, wc, bc, we, be, strength)
    res = run_bass_kernel_spmd(nc, in_maps, core_ids=list(range(8)))
    return _assemble(res.results)


def kernel_traced(hidden_states, wc, bc, we, be, strength, key=None,
                  **trace_kwargs):
    """Test-harness entry: returns (output, BassKernelResults with trace)."""
    nc = _get_nc()
    in_maps = _prep_in_maps(hidden_states, wc, bc, we, be, strength)
    res = run_bass_kernel_spmd(nc, in_maps, core_ids=list(range(8)),
                               trace=True, **trace_kwargs)
    return _assemble(res.results), res
